# revision 34
# baseline (speedup 1.0000x reference)
"""Trainium2 Bass kernel for heterogeneous GNN (GAT + FFN), v2.

One NEFF on NCORES=8 cores:

  Phase A (replicated): host pre-gathers embedding rows into node order,
  transposed [D, nodes] bf16.  Device streams xT, computes h = W''^T x in
  PSUM ([HID, 512 nodes]), copies to bf16, then ONE matmul per 128-node
  block with rhs = [I | AL | AR] produces the row-major H block
  [node, h(128) | el(8) | er(8)] directly (no tensor transposes).  H rows
  are laid out p-major inside each 512 block (row = q*512 + p*4 + g) so the
  SBUF->HBM write is 1152B/partition descriptors (full DMA rate).

  Phase B (edges sharded by dst window across 8 cores): edges with sentence
  dst, sorted by dst, per-window slabs padded to the cross-core max so one
  NEFF serves all cores.  Per 16-slab chunk: 16 indirect gathers of H[src];
  one-hot masks (host precomputed, DMA'd): mT (dst-major) for per-edge er
  via matmul, m_t (edge-major) for segment-sum; batched z/leaky/exp chain;
  per-slab psw accumulation; per-4-window batched softmax-normalize + ELU +
  FFN (residual) + logits.
"""

import os
import numpy as np

import concourse.bacc as bacc
import concourse.bass as bass
import concourse.mybir as mybir
import concourse.tile as tile


def _run_spmd(nc, in_maps, n_cores=8, bench=0):
    """Execute a compiled Bass program on n_cores via PJRT (axon).

    Pre-places inputs on device so repeated timed calls measure NEFF
    execution only.  Returns (results_per_core, best_exec_seconds or None).
    """
    import time as _time
    import jax
    from jax.sharding import Mesh, PartitionSpec, NamedSharding
    from jax.experimental.shard_map import shard_map
    from concourse import bass2jax as b2j
    from concourse import mybir as mb

    b2j.install_neuronx_cc_hook()
    part_name = nc.partition_id_tensor.name if nc.partition_id_tensor else None
    in_names, out_names, out_avals, zero_outs = [], [], [], []
    for alloc in nc.m.functions[0].allocations:
        if not isinstance(alloc, mb.MemoryLocationSet):
            continue
        name = alloc.memorylocations[0].name
        if alloc.kind == "ExternalInput":
            if name != part_name:
                in_names.append(name)
        elif alloc.kind == "ExternalOutput":
            out_names.append(name)
            shape = tuple(alloc.tensor_shape)
            dtype = mb.dt.np(alloc.dtype)
            out_avals.append(jax.core.ShapedArray(shape, dtype))
            zero_outs.append(np.zeros(shape, dtype))
    n_params = len(in_names)
    n_outs = len(out_avals)
    all_names = in_names + out_names
    if part_name is not None:
        all_names = all_names + [part_name]

    def _body(*args):
        operands = list(args)
        if part_name is not None:
            operands.append(b2j.partition_id_tensor())
        outs = b2j._bass_exec_p.bind(
            *operands,
            out_avals=tuple(out_avals),
            in_names=tuple(all_names),
            out_names=tuple(out_names),
            lowering_input_output_aliases=(),
            sim_require_finite=True,
            sim_require_nnan=True,
            nc=nc,
        )
        return tuple(outs)

    devices = jax.devices()[:n_cores]
    mesh = Mesh(np.asarray(devices), ("core",))
    donate = tuple(range(n_params, n_params + n_outs))
    sharded = jax.jit(
        shard_map(_body, mesh=mesh,
                  in_specs=(PartitionSpec("core"),) * (n_params + n_outs),
                  out_specs=(PartitionSpec("core"),) * n_outs,
                  check_rep=False),
        donate_argnums=donate, keep_unused=True)
    spec = NamedSharding(mesh, PartitionSpec("core"))
    concat_in = [
        jax.device_put(
            np.concatenate([np.asarray(in_maps[c][nm]) for c in range(n_cores)],
                           axis=0), spec)
        for nm in in_names
    ]
    def _zeros():
        return [jax.device_put(
                    np.zeros((n_cores * z.shape[0], *z.shape[1:]), z.dtype),
                    spec)
                for z in zero_outs]

    out_arrs = sharded(*concat_in, *_zeros())
    jax.block_until_ready(out_arrs)
    results = [
        {nm: np.asarray(out_arrs[i]).reshape(n_cores, *out_avals[i].shape)[c]
         for i, nm in enumerate(out_names)}
        for c in range(n_cores)
    ]
    ntff_dir = os.environ.get("KERNEL_NTFF_DIR")
    if ntff_dir:
        import ctypes
        lib = ctypes.CDLL("/opt/axon/libaxon_pjrt.so")
        lib.axon_start_nrt_profile.argtypes = [
            ctypes.POINTER(ctypes.c_int64), ctypes.c_size_t]
        lib.axon_start_nrt_profile.restype = ctypes.c_int64
        lib.axon_stop_nrt_profile.argtypes = [ctypes.c_char_p]
        lib.axon_stop_nrt_profile.restype = ctypes.c_int64
        rc = lib.axon_start_nrt_profile(None, 0)
        if rc != 0:
            raise RuntimeError(f"axon_start_nrt_profile rc={rc}")
        zz = _zeros()
        jax.block_until_ready(zz)
        oo = sharded(*concat_in, *zz)
        jax.block_until_ready(oo)
        n = lib.axon_stop_nrt_profile(ntff_dir.encode())
        print(f"ntff: {n} file(s) -> {ntff_dir}")
    best = None
    for _ in range(bench):
        zz = _zeros()
        jax.block_until_ready(zz)
        t0 = _time.perf_counter()
        oo = sharded(*concat_in, *zz)
        jax.block_until_ready(oo)
        dt_s = _time.perf_counter() - t0
        best = dt_s if best is None or dt_s < best else best
    # pipelined amortized timing: N back-to-back executes, one sync
    npipe = int(os.environ.get("KERNEL_PIPE", "0"))
    amort = None
    if npipe > 1:
        zzs = [_zeros() for _ in range(npipe)]
        jax.block_until_ready(zzs)
        t0 = _time.perf_counter()
        oos = [sharded(*concat_in, *zz) for zz in zzs]
        jax.block_until_ready(oos)
        amort = (_time.perf_counter() - t0) / npipe
    return results, best, amort


N_FEAT, N_SENT, N_USER, N_ITEM = 60000, 100000, 5000, 5000
FEAT_NUM, SENT_NUM, USER_NUM, ITEM_NUM = 60000, 200000, 100000, 50000
HEADS, DH, HID = 8, 16, 128
FFN = 512
ROW = 144                     # h(128) | el(8) | er(8)
NCORES = 8
SUP = 2048                    # nodes per phase-A superblock
CHUNK = 16                    # slabs per phase-B chunk (2048 edges)

F32 = mybir.dt.float32
BF16 = mybir.dt.bfloat16
I32 = mybir.dt.int32

NW_TOT = (N_SENT + 127) // 128            # 782 real dst windows
NW_CORE = (NW_TOT + NCORES - 1) // NCORES  # local windows per core
P_S_CORE = NW_CORE * 128                  # logits per core

# per-type D-chunk heights (true dims, no padding of the contraction)
TYPE_CHUNKS = {"s": [128] * 6, "f": [128, 128, 44], "u": [64], "i": [64]}
TYPE_D = {"s": 768, "f": 300, "u": 64, "i": 64}

LAST_STATS = {}


def _ru(x, m):
    return ((x + m - 1) // m) * m


def _build(n_slabs, slab_win, pc, kind):
    """One NEFF: phase A (per-core compacted projection) + phase B (edges).

    pc maps type -> padded per-core node count (multiple of SUP).  H-row
    segment order is f | u | i | s, with the s segment holding the core's
    NW_CORE dst windows first (fixed rows for er loads) then its remaining
    sentence src nodes.

    kind[slab] is True for "early" slabs whose 128 src rows are all in the
    f/u/i region: those gather from the small duplicate table Hto (written
    before the sentence block) into a DRAM staging area DURING phase A's
    sentence block, hiding their ~1us/slab GpSimd descriptor-gen cost.
    """
    nc = bacc.Bacc("TRN2", target_bir_lowering=False, debug=False,
                   enable_asserts=False)

    FB2 = 0
    UB2 = FB2 + pc["f"]
    IB2 = UB2 + pc["u"]
    SB2 = IB2 + pc["i"]
    R_TOT = SB2 + pc["s"]
    types = [("f", pc["f"], FB2), ("u", pc["u"], UB2),
             ("i", pc["i"], IB2), ("s", pc["s"], SB2)]
    early_ids = [s for s in range(n_slabs) if kind[s]]
    early_of = {s: i for i, s in enumerate(early_ids)}
    n_early = len(early_ids)

    xs_d, wp_d = {}, {}
    for t, P_t, base in types:
        D = TYPE_D[t]
        xs_d[t] = nc.dram_tensor(f"xs_{t}", [D, P_t], BF16,
                                 kind="ExternalInput").ap()
        wp_d[t] = nc.dram_tensor(f"wp_{t}", [D, HID], BF16,
                                 kind="ExternalInput").ap()
    iaa_d = nc.dram_tensor("iaa", [HID, ROW], BF16, kind="ExternalInput").ap()
    identf_d = nc.dram_tensor("identf", [128, 128], F32, kind="ExternalInput").ap()
    src_d = nc.dram_tensor("src", [128, n_slabs], I32, kind="ExternalInput").ap()
    srce_d = nc.dram_tensor("srce", [128, max(n_early, 1)], I32,
                            kind="ExternalInput").ap()
    mte_d = nc.dram_tensor("mte", [128, n_slabs * 128], BF16,
                           kind="ExternalInput").ap()
    mtd_d = nc.dram_tensor("mtd", [128, n_slabs * 128], BF16,
                           kind="ExternalInput").ap()
    w1_d = nc.dram_tensor("w1", [HID, FFN], BF16, kind="ExternalInput").ap()
    b1_d = nc.dram_tensor("b1c", [128, FFN // 128], F32, kind="ExternalInput").ap()
    w2_d = nc.dram_tensor("w2", [FFN, HID], BF16, kind="ExternalInput").ap()
    b2_d = nc.dram_tensor("b2c", [128, 1], F32, kind="ExternalInput").ap()
    wh_d = nc.dram_tensor("wh", [HID, 1], BF16, kind="ExternalInput").ap()
    whb_d = nc.dram_tensor("whb", [1, 1], F32, kind="ExternalInput").ap()
    lg_d = nc.dram_tensor("logits", [1, P_S_CORE], F32, kind="ExternalOutput").ap()

    H_d = nc.dram_tensor("Htab", [R_TOT, ROW], BF16, kind="Internal").ap()
    Hto_d = nc.dram_tensor("Hto", [SB2, ROW], BF16, kind="Internal").ap()
    GE_d = (nc.dram_tensor("GE", [max(n_early, 1) * 128, ROW], BF16,
                           kind="Internal").ap()
            if n_early else None)

    first_slab, last_slab = {}, {}
    for s, w in enumerate(slab_win):
        first_slab.setdefault(w, s)
        last_slab[w] = s
    n_chunks = n_slabs // CHUNK

    with tile.TileContext(nc) as tc:
        # ------------------------------------------------------------------
        # phase A: stream xT, project, write H rows
        # ------------------------------------------------------------------
        with (
            tc.tile_pool(name="acst", bufs=1) as acpool,
            tc.tile_pool(name="xt", bufs=2) as xtpool,
            tc.tile_pool(name="ht", bufs=3) as htpool,
            tc.tile_pool(name="hrow", bufs=3) as hrowpool,
            tc.tile_pool(name="ge", bufs=3) as gepool,
            tc.tile_pool(name="ps_h", bufs=3, space="PSUM") as ps_h,
            tc.tile_pool(name="ps_o", bufs=3, space="PSUM") as ps_o,
        ):
            src_eb = acpool.tile([128, max(n_early, 1)], I32)
            nc.sync.dma_start(src_eb[:], srce_d[:])

            def early_pass():
                # gather all-f/u/i slabs from Hto into GE staging while the
                # tensor engine chews on the sentence block
                for e0 in range(0, n_early, 16):
                    ne = min(16, n_early - e0)
                    ge = gepool.tile([128, 16 * ROW], BF16, tag="ge")
                    for k in range(ne):
                        nc.gpsimd.indirect_dma_start(
                            out=ge[:, k * ROW:(k + 1) * ROW], out_offset=None,
                            in_=Hto_d[:],
                            in_offset=bass.IndirectOffsetOnAxis(
                                ap=src_eb[:, e0 + k:e0 + k + 1], axis=0))
                    # SWDGE write: keeps the staging store on the Pool queue
                    # (in-order Sync would head-of-line block the sentence
                    # xs loads behind these gather-dependent stores)
                    nc.gpsimd.dma_start(
                        out=GE_d[e0 * 128:(e0 + ne) * 128, :].rearrange(
                            "(s p) r -> p s r", p=128),
                        in_=ge[:, 0:ne * ROW].rearrange("p (s r) -> p s r", s=ne))
            iaa = acpool.tile([HID, ROW], BF16)
            nc.sync.dma_start(iaa[:], iaa_d[:])
            wp_sb = {}
            for t, P_t, base in types:
                cks = TYPE_CHUNKS[t]
                wp_sb[t] = acpool.tile([128, len(cks) * HID], BF16,
                                       tag=f"wp{t}", name=f"wp_sb_{t}")
                c0 = 0
                for ci, h in enumerate(cks):
                    nc.sync.dma_start(
                        wp_sb[t][0:h, ci * HID:(ci + 1) * HID],
                        wp_d[t][c0:c0 + h, :])
                    c0 += h

            for t, P_t, base in types:
                if t == "s" and n_early:
                    early_pass()
                cks = TYPE_CHUNKS[t]
                nck = len(cks)
                for sbk in range(P_t // SUP):
                    xt = xtpool.tile([128, 6 * SUP], BF16,
                                     tag="xt", name=f"xt_{t}_{sbk}")
                    c0 = 0
                    for ci, h in enumerate(cks):
                        nc.sync.dma_start(
                            xt[0:h, ci * SUP:(ci + 1) * SUP],
                            xs_d[t][c0:c0 + h, sbk * SUP:(sbk + 1) * SUP])
                        c0 += h
                    for q in range(SUP // 512):
                        psh = ps_h.tile([128, 512], F32, tag="psh")
                        for ci, h in enumerate(cks):
                            rhs = xt[0:h, ci * SUP + q * 512:
                                     ci * SUP + q * 512 + 512]
                            nc.tensor.matmul(psh[:],
                                             wp_sb[t][0:h, ci * HID:(ci + 1) * HID],
                                             rhs, start=(ci == 0),
                                             stop=(ci == nck - 1))
                        ht = htpool.tile([128, 512], BF16, tag="ht")
                        nc.scalar.activation(ht[:], psh[:],
                                             mybir.ActivationFunctionType.Identity)
                        hrow = hrowpool.tile([128, 4 * ROW], BF16, tag="hrow")
                        for half in range(2):
                            pso = ps_o.tile([128, 2 * ROW], F32, tag="pso")
                            for k in range(2):
                                g = half * 2 + k
                                nc.tensor.matmul(
                                    pso[:, k * ROW:(k + 1) * ROW],
                                    ht[:, g * 128:(g + 1) * 128],
                                    iaa[:], start=True, stop=True)
                            nc.vector.tensor_copy(
                                hrow[:, half * 2 * ROW:(half + 1) * 2 * ROW],
                                pso[:])
                        r0 = base + sbk * SUP + q * 512
                        nc.sync.dma_start(
                            H_d[r0:r0 + 512, :].rearrange(
                                "(p g) r -> p (g r)", p=128, g=4),
                            hrow[:])
                        if t != "s" and n_early:
                            nc.sync.dma_start(
                                Hto_d[r0:r0 + 512, :].rearrange(
                                    "(p g) r -> p (g r)", p=128, g=4),
                                hrow[:])

        # ------------------------------------------------------------------
        # phase B: edge aggregation + FFN + logits
        # ------------------------------------------------------------------
        with (
            tc.tile_pool(name="const", bufs=1) as cpool,
            tc.tile_pool(name="g", bufs=6) as gpool,
            tc.tile_pool(name="t", bufs=3) as tpool,
            tc.tile_pool(name="me", bufs=3) as mepool,
            tc.tile_pool(name="md", bufs=3) as mdpool,
            tc.tile_pool(name="sm", bufs=3) as smpool,
            tc.tile_pool(name="er", bufs=3) as erpool,
            tc.tile_pool(name="ev", bufs=2) as evpool,
            tc.tile_pool(name="xh", bufs=2) as xhpool,
            tc.tile_pool(name="xf", bufs=2) as xfpool,
            tc.tile_pool(name="y", bufs=2) as ypool,
            tc.tile_pool(name="lgt", bufs=2) as lgpool,
            tc.tile_pool(name="ps_w", bufs=2, space="PSUM") as ps_w,
            tc.tile_pool(name="ps_er", bufs=2, space="PSUM") as ps_er,
            tc.tile_pool(name="ps_tp", bufs=1, space="PSUM") as ps_tp,
            tc.tile_pool(name="ps_y", bufs=1, space="PSUM") as ps_y,
            tc.tile_pool(name="ps_z", bufs=1, space="PSUM") as ps_z,
            tc.tile_pool(name="ps_l", bufs=1, space="PSUM") as ps_l,
        ):
            identf = cpool.tile([128, 128], F32)
            w1_t = cpool.tile([HID, FFN], BF16)
            b1_t = cpool.tile([128, FFN // 128], F32)
            w2_t = cpool.tile([128, FFN], BF16)  # block j = W2[j*128:(j+1)*128,:]
            b2_t = cpool.tile([128, 1], F32)
            wh_t = cpool.tile([HID, 1], BF16)
            whb_t = cpool.tile([1, 1], F32)
            src_sb = cpool.tile([128, n_slabs], I32)
            nc.sync.dma_start(identf[:], identf_d[:])
            nc.sync.dma_start(w1_t[:], w1_d[:])
            nc.sync.dma_start(b1_t[:], b1_d[:])
            for j in range(FFN // 128):
                nc.sync.dma_start(w2_t[:, j * 128:(j + 1) * 128],
                                  w2_d[j * 128:(j + 1) * 128, :])
            nc.sync.dma_start(b2_t[:], b2_d[:])
            nc.sync.dma_start(wh_t[:], wh_d[:])
            nc.sync.dma_start(whb_t[:], whb_d[:])
            nc.sync.dma_start(src_sb[:], src_d[:])

            win_psum = {}
            win_er = {}
            grp = {"ev": None, "count": 0, "base": 0}

            def flush_group():
                K = grp["count"]
                if K == 0:
                    return
                ev = grp["ev"]
                nb = K * 128
                evv = ev[:].rearrange("p (k c) -> p k c", k=4)
                den = evpool.tile([128, 32], F32, tag="den")
                nc.vector.tensor_scalar(
                    out=den[:, 0:K * 8].rearrange("p (k h) -> p k h", k=K),
                    in0=evv[:, 0:K, HID:HID + 8],
                    scalar1=1e-9, scalar2=None,
                    op0=mybir.AluOpType.add)
                rcp = evpool.tile([128, 32], F32, tag="rcp")
                nc.vector.reciprocal(rcp[:, 0:K * 8], den[:, 0:K * 8])
                xh = xhpool.tile([128, 512], F32, tag="xh")
                nc.vector.tensor_tensor(
                    out=xh[:, 0:nb].rearrange("p (k h r) -> p k h r", k=K, h=8),
                    in0=evv[:, 0:K, 0:HID].rearrange(
                        "p k (h r) -> p k h r", h=8),
                    in1=rcp[:, 0:K * 8].rearrange("p (k h) -> p k h", k=K)
                        .unsqueeze(3).broadcast_to([128, K, 8, 16]),
                    op=mybir.AluOpType.mult)
                neg = evpool.tile([128, 512], F32, tag="neg")
                nc.vector.tensor_scalar(out=neg[:, 0:nb], in0=xh[:, 0:nb],
                                        scalar1=0.0, scalar2=None,
                                        op0=mybir.AluOpType.min)
                emn = evpool.tile([128, 512], F32, tag="emn")
                nc.scalar.activation(emn[:, 0:nb], neg[:, 0:nb],
                                     mybir.ActivationFunctionType.Exp)
                nc.vector.tensor_scalar(out=emn[:, 0:nb], in0=emn[:, 0:nb],
                                        scalar1=-1.0, scalar2=None,
                                        op0=mybir.AluOpType.add)
                pos = evpool.tile([128, 512], F32, tag="pos")
                nc.vector.tensor_scalar(out=pos[:, 0:nb], in0=xh[:, 0:nb],
                                        scalar1=0.0, scalar2=None,
                                        op0=mybir.AluOpType.max)
                elu = evpool.tile([128, 512], F32, tag="elu")
                nc.vector.tensor_tensor(out=elu[:, 0:nb], in0=pos[:, 0:nb],
                                        in1=emn[:, 0:nb],
                                        op=mybir.AluOpType.add)
                pst = ps_tp.tile([128, 512], F32, tag="pstp")
                for k in range(K):
                    nc.tensor.transpose(out=pst[:, k * 128:(k + 1) * 128],
                                        in_=elu[:, k * 128:(k + 1) * 128],
                                        identity=identf[:])
                xf = xfpool.tile([128, 512], F32, tag="xf")
                nc.vector.tensor_copy(xf[:, 0:nb], pst[:, 0:nb])
                xfr = ypool.tile([128, 512], BF16, tag="xfr")
                nc.scalar.activation(xfr[:, 0:nb], pst[:, 0:nb],
                                     mybir.ActivationFunctionType.Identity)
                yts = []
                for j in range(FFN // 128):
                    psy = ps_y.tile([128, 512], F32, tag="psy")
                    nc.tensor.matmul(
                        psy[:, 0:nb],
                        w1_t[:, j * 128:(j + 1) * 128],
                        xfr[:, 0:nb],
                        start=True, stop=True)
                    y_t = ypool.tile([128, 512], BF16, tag=f"y{j}")
                    nc.scalar.activation(y_t[:, 0:nb], psy[:, 0:nb],
                                         mybir.ActivationFunctionType.Relu,
                                         bias=b1_t[:, j:j + 1])
                    yts.append(y_t)
                psz = ps_z.tile([128, 512], F32, tag="psz")
                for j in range(FFN // 128):
                    nc.tensor.matmul(
                        psz[:, 0:nb],
                        w2_t[:, j * 128:(j + 1) * 128],
                        yts[j][:, 0:nb],
                        start=(j == 0), stop=(j == FFN // 128 - 1))
                z_t = lgpool.tile([128, 512], F32, tag="z")
                nc.scalar.activation(z_t[:, 0:nb], psz[:, 0:nb],
                                     mybir.ActivationFunctionType.Identity,
                                     bias=b2_t[:, 0:1])
                nc.vector.tensor_tensor(out=z_t[:, 0:nb], in0=z_t[:, 0:nb],
                                        in1=xf[:, 0:nb], op=mybir.AluOpType.add)
                zb_t = lgpool.tile([128, 512], BF16, tag="zb")
                nc.vector.tensor_copy(zb_t[:, 0:nb], z_t[:, 0:nb])
                psl = ps_l.tile([1, 512], F32, tag="psl")
                nc.tensor.matmul(psl[0:1, 0:nb], wh_t[:], zb_t[:, 0:nb],
                                 start=True, stop=True)
                lg_t = lgpool.tile([1, 512], F32, tag="lg")
                nc.scalar.activation(lg_t[0:1, 0:nb], psl[0:1, 0:nb],
                                     mybir.ActivationFunctionType.Identity,
                                     bias=whb_t[0:1, 0:1])
                b0 = grp["base"] * 128
                nc.sync.dma_start(lg_d[0:1, b0:b0 + nb], lg_t[0:1, 0:nb])
                grp["ev"] = None
                grp["count"] = 0

            def evacuate(w):
                psw = win_psum.pop(w)
                win_er.pop(w, None)
                if grp["ev"] is None:
                    grp["ev"] = evpool.tile([128, 4 * 136], F32, tag="ev4",
                                            name=f"ev4_{w}")
                    grp["base"] = w
                k = grp["count"]
                nc.vector.tensor_copy(grp["ev"][:, k * 136:(k + 1) * 136],
                                      psw[:])
                grp["count"] = k + 1
                if grp["count"] == 4:
                    flush_group()

            def load_chunk(c, g_t):
                g = 0
                while g < CHUNK:
                    sc = c * CHUNK + g
                    if kind[sc]:
                        # maximal run of early slabs: one staged read from GE
                        g1 = g
                        while g1 < CHUNK and kind[c * CHUNK + g1]:
                            g1 += 1
                        e0 = early_of[sc]
                        ne = g1 - g
                        nc.sync.dma_start(
                            g_t[:, g * ROW:g1 * ROW].rearrange(
                                "p (s r) -> p s r", s=ne),
                            GE_d[e0 * 128:(e0 + ne) * 128, :].rearrange(
                                "(s p) r -> p s r", p=128))
                        g = g1
                    else:
                        nc.gpsimd.indirect_dma_start(
                            out=g_t[:, g * ROW:(g + 1) * ROW], out_offset=None,
                            in_=H_d[:],
                            in_offset=bass.IndirectOffsetOnAxis(
                                ap=src_sb[:, sc:sc + 1], axis=0))
                        g += 1

            for c in range(n_chunks):
                g_t = gpool.tile([128, CHUNK * ROW], BF16, tag="g")
                load_chunk(c, g_t)
                gv = g_t[:].rearrange("p (g r) -> p g r", g=CHUNK)
                mte = mepool.tile([128, CHUNK * 128], BF16, tag="mte")
                nc.sync.dma_start(
                    mte[:], mte_d[:, c * CHUNK * 128:(c + 1) * CHUNK * 128])
                mtd = mdpool.tile([128, CHUNK * 128], BF16, tag="mtd")
                nc.sync.dma_start(
                    mtd[:], mtd_d[:, c * CHUNK * 128:(c + 1) * CHUNK * 128])
                # per-edge er via mT @ er_win, batched into one PSUM tile
                pser = ps_er.tile([128, CHUNK * 8], F32, tag="pser")
                for g in range(CHUNK):
                    sc = c * CHUNK + g
                    w = slab_win[sc]
                    if w not in win_er:
                        ert = erpool.tile([128, 8], BF16, tag="er",
                                          name=f"er_{w}")
                        rs = SB2 + (w // 4) * 512
                        g0 = w % 4
                        nc.sync.dma_start(
                            ert[:].rearrange("p (a c) -> p a c", a=1),
                            H_d[rs:rs + 512, HID + 8:ROW].rearrange(
                                "(p g) c -> p g c", p=128, g=4)
                            [:, g0:g0 + 1, :])
                        win_er[w] = ert
                    nc.tensor.matmul(pser[:, g * 8:(g + 1) * 8],
                                     mtd[:, g * 128:(g + 1) * 128],
                                     win_er[w][:], start=True, stop=True)
                # z = el[src] + er[dst]; s = exp(leaky_relu(z))
                z_t = smpool.tile([128, CHUNK * 8], F32, tag="z8")
                nc.vector.tensor_tensor(
                    out=z_t[:].rearrange("p (g h) -> p g h", g=CHUNK),
                    in0=gv[:, :, HID:HID + 8],
                    in1=pser[:].rearrange("p (g h) -> p g h", g=CHUNK),
                    op=mybir.AluOpType.add)
                zz_t = smpool.tile([128, CHUNK * 8], F32, tag="zz8")
                nc.scalar.mul(zz_t[:], z_t[:], 0.2)
                nc.vector.tensor_tensor(out=z_t[:], in0=z_t[:], in1=zz_t[:],
                                        op=mybir.AluOpType.max)
                sb_t = smpool.tile([128, CHUNK * 8], BF16, tag="sb8")
                nc.scalar.activation(sb_t[:], z_t[:],
                                     mybir.ActivationFunctionType.Exp)
                sv = sb_t[:].rearrange("p (g h) -> p g h", g=CHUNK)
                t_t = tpool.tile([128, CHUNK * 136], BF16, tag="t")
                tv = t_t[:].rearrange("p (g c) -> p g c", g=CHUNK)
                nc.vector.tensor_tensor(
                    out=tv[:, :, 0:HID].rearrange("p g (h r) -> p g h r", h=8),
                    in0=gv[:, :, 0:HID].rearrange("p g (h r) -> p g h r", h=8),
                    in1=sv.unsqueeze(3).broadcast_to([128, CHUNK, 8, 16]),
                    op=mybir.AluOpType.mult)
                nc.vector.tensor_copy(tv[:, :, HID:HID + 8], sv)
                for s in range(CHUNK):
                    gs = c * CHUNK + s
                    w = slab_win[gs]
                    if w not in win_psum:
                        win_psum[w] = ps_w.tile([128, 136], F32, tag="psw",
                                                name=f"psw_{w}")
                    nc.tensor.matmul(
                        win_psum[w][:],
                        mte[:, s * 128:(s + 1) * 128],
                        t_t[:, s * 136:(s + 1) * 136],
                        start=(gs == first_slab[w]), stop=(gs == last_slab[w]))
                    if gs == last_slab[w]:
                        evacuate(w)
            flush_group()
    nc.compile()
    return nc


# ----------------------------------------------------------------------------
# host orchestration
# ----------------------------------------------------------------------------

def _row_of_block(n, base):
    """p-major H-row layout inside each 512 block: row = q*512 + p*4 + g."""
    q, j = n // 512, n % 512
    g, p = j // 128, j % 128
    return base + q * 512 + p * 4 + g


def kernel(**inputs):
    global LAST_STATS
    LAST_STATS = {}
    import ml_dtypes
    bf16 = ml_dtypes.bfloat16
    bench = int(os.environ.get("KERNEL_BENCH", "0"))

    fid = np.asarray(inputs["fid"]).astype(np.int64)
    sid = np.asarray(inputs["sid"]).astype(np.int64)
    uids = np.asarray(inputs["uids"]).astype(np.int64)
    iids = np.asarray(inputs["iids"]).astype(np.int64)
    src = np.asarray(inputs["src"]).astype(np.int64)
    dst = np.asarray(inputs["dst"]).astype(np.int64)

    Wg = np.asarray(inputs["Wg"], dtype=np.float32)
    attn_l = np.asarray(inputs["attn_l"], dtype=np.float32)
    attn_r = np.asarray(inputs["attn_r"], dtype=np.float32)
    AL = np.zeros((HID, HEADS), dtype=np.float32)
    AR = np.zeros((HID, HEADS), dtype=np.float32)
    for h in range(HEADS):
        AL[h * DH:(h + 1) * DH, h] = attn_l[h]
        AR[h * DH:(h + 1) * DH, h] = attn_r[h]
    iaa_np = np.concatenate([np.eye(HID, dtype=np.float32), AL, AR],
                            axis=1).astype(bf16)          # [128,144]

    # per-type node feature rows (node order, f32) + W' = Wt @ Wg (bf16)
    tabs = {
        "s": (np.asarray(inputs["sent_embed"], dtype=np.float32), sid),
        "f": (np.asarray(inputs["feature_embed"], dtype=np.float32), fid),
        "u": (np.asarray(inputs["user_embed"], dtype=np.float32), uids),
        "i": (np.asarray(inputs["item_embed"], dtype=np.float32), iids),
    }
    wp_np = {
        t: (np.asarray(inputs["W" + t], dtype=np.float32) @ Wg).astype(bf16)
        for t in ("s", "f", "u", "i")
    }

    # ---- edges (sentence dst only), sharded by dst window ----
    B_S, B_U, B_I = N_FEAT, N_FEAT + N_SENT, N_FEAT + N_SENT + N_USER
    keep = (dst >= N_FEAT) & (dst < N_FEAT + N_SENT)
    e_src = src[keep]
    e_d = dst[keep] - N_FEAT
    e_w = e_d // 128
    e_ns = (e_src < B_S) | (e_src >= B_U)    # non-sentence src (f/u/i)

    # balanced window->core assignment: sort global windows by edge count,
    # deal each group-of-8 across the 8 cores.  Minimizes the cross-core max
    # that the shared-NEFF per-window slab padding pays.  2 phantom empty
    # windows pad 782 -> 784 = 8*98.
    NWP = NCORES * NW_CORE
    wcnt = np.bincount(e_w, minlength=NWP)
    worder = np.argsort(-wcnt, kind="stable")
    wperm = np.empty((NCORES, NW_CORE), dtype=np.int64)   # (core,slot)->gw
    wcore = np.empty(NWP, dtype=np.int64)
    wslot = np.empty(NWP, dtype=np.int64)
    for k in range(NW_CORE):
        for c in range(NCORES):
            gw = int(worder[k * NCORES + c])
            wperm[c, k] = gw
            wcore[gw] = c
            wslot[gw] = k
    e_core = wcore[e_w]
    e_dl = wslot[e_w] * 128 + (e_d % 128)    # core-local dst index

    core_sorted = []
    cnt_w = np.zeros((NCORES, NW_CORE), dtype=np.int64)
    ns_w = np.zeros((NCORES, NW_CORE), dtype=np.int64)
    for c in range(NCORES):
        sel = np.where(e_core == c)[0]
        # group by dst slot; non-sentence-src edges first within each window
        o = np.lexsort(((~e_ns[sel]).astype(np.int64), e_dl[sel] // 128))
        sel = sel[o]
        dl = e_dl[sel]
        slot = dl // 128
        wstart = np.searchsorted(slot, np.arange(NW_CORE + 1))
        for w in range(NW_CORE):
            a, b = int(wstart[w]), int(wstart[w + 1])
            cnt_w[c, w] = _ru(max(b - a, 1), 128) // 128
            ns_w[c, w] = int(np.count_nonzero(e_ns[sel[a:b]]))
        core_sorted.append((sel, dl, wstart))
    req = cnt_w.max(axis=0)
    SLABS = _ru(int(req.sum()), CHUNK)
    req[NW_CORE - 1] += SLABS - int(req.sum())
    slab_win = []
    for w in range(NW_CORE):
        slab_win.extend([w] * int(req[w]))
    # early slabs: the first min_c floor(ns/128) slabs of each window are
    # all-f/u/i-src on EVERY core -> gatherable from Hto during phase A
    early_req = (ns_w // 128).min(axis=0)
    kind = np.zeros(SLABS, dtype=bool)
    s0 = 0
    for w in range(NW_CORE):
        kind[s0:s0 + int(early_req[w])] = True
        s0 += int(req[w])

    # ---- per-core node compaction: each core only projects the nodes it
    # actually touches (src of its edges + its own dst-window sentence nodes)
    core_nodes = []
    for c in range(NCORES):
        sel, dl, wstart = core_sorted[c]
        es = e_src[sel]
        f_ids = np.unique(es[es < B_S])
        s_src = np.unique(es[(es >= B_S) & (es < B_U)]) - B_S
        u_ids = np.unique(es[(es >= B_U) & (es < B_I)]) - B_U
        i_ids = np.unique(es[es >= B_I]) - B_I
        s_extra = s_src[wcore[s_src // 128] != c]
        core_nodes.append((f_ids, s_extra, u_ids, i_ids))
    pc = {
        "f": _ru(max(1, max(len(cn[0]) for cn in core_nodes)), SUP),
        "s": _ru(P_S_CORE + max(len(cn[1]) for cn in core_nodes), SUP),
        "u": _ru(max(1, max(len(cn[2]) for cn in core_nodes)), SUP),
        "i": _ru(max(1, max(len(cn[3]) for cn in core_nodes)), SUP),
    }
    FB2 = 0
    UB2 = FB2 + pc["f"]
    IB2 = UB2 + pc["u"]
    SB2 = IB2 + pc["i"]

    core_edges = []
    for c in range(NCORES):
        sel, dl, wstart = core_sorted[c]
        f_ids, s_extra, u_ids, i_ids = core_nodes[c]
        # compact index per global node id (only valid for this core's nodes)
        cmp_f = np.zeros(N_FEAT, dtype=np.int64)
        cmp_f[f_ids] = np.arange(len(f_ids))
        cmp_s = np.zeros(N_SENT, dtype=np.int64)
        # window region: slot k holds global window wperm[c,k]'s 128 nodes
        wnode = (wperm[c][:, None] * 128
                 + np.arange(128)[None, :]).ravel()       # [P_S_CORE]
        wvalid = wnode < N_SENT
        cmp_s[wnode[wvalid]] = np.arange(P_S_CORE)[wvalid]
        cmp_s[s_extra] = P_S_CORE + np.arange(len(s_extra))
        cmp_u = np.zeros(N_USER, dtype=np.int64)
        cmp_u[u_ids] = np.arange(len(u_ids))
        cmp_i = np.zeros(N_ITEM, dtype=np.int64)
        cmp_i[i_ids] = np.arange(len(i_ids))

        es = e_src[sel]
        sr = np.empty(len(es), dtype=np.int64)
        mf = es < B_S
        ms = (es >= B_S) & (es < B_U)
        mu = (es >= B_U) & (es < B_I)
        mi = es >= B_I
        sr[mf] = _row_of_block(cmp_f[es[mf]], FB2)
        sr[ms] = _row_of_block(cmp_s[es[ms] - B_S], SB2)
        sr[mu] = _row_of_block(cmp_u[es[mu] - B_U], UB2)
        sr[mi] = _row_of_block(cmp_i[es[mi] - B_I], IB2)

        sw_l, rw_l = [], []
        for w in range(NW_CORE):
            a, b = int(wstart[w]), int(wstart[w + 1])
            n = b - a
            npad = int(req[w]) * 128
            sw = np.zeros(npad, dtype=np.int32)
            rw = np.full(npad, -1.0, dtype=np.float32)
            sw[:n] = sr[a:b]
            rw[:n] = (dl[a:b] % 128).astype(np.float32)
            sw_l.append(sw)
            rw_l.append(rw)
        sw = np.concatenate(sw_l).reshape(SLABS, 128)
        rw = np.concatenate(rw_l).reshape(SLABS, 128)
        # one-hot masks: M[s, e, d] = (rw[s,e] == d)
        M = (rw[:, :, None] == np.arange(128, dtype=np.float32)[None, None, :])
        mte = np.ascontiguousarray(
            M.transpose(1, 0, 2).reshape(128, SLABS * 128)).astype(bf16)
        mtd = np.ascontiguousarray(
            M.transpose(2, 0, 1).reshape(128, SLABS * 128)).astype(bf16)

        # per-core compacted xT inputs
        xs_c = {}
        for t, ids_local in (("f", f_ids), ("u", u_ids), ("i", i_ids)):
            tab, gids = tabs[t]
            D = tab.shape[1]
            xT = np.zeros((D, pc[t]), dtype=bf16)
            if len(ids_local):
                xT[:, :len(ids_local)] = tab[gids[ids_local]].T.astype(bf16)
            xs_c[t] = xT
        tab, gids = tabs["s"]
        xT = np.zeros((768, pc["s"]), dtype=bf16)
        cols = np.where(wvalid)[0]
        xT[:, cols] = tab[gids[wnode[wvalid]]].T.astype(bf16)
        if len(s_extra):
            xT[:, P_S_CORE:P_S_CORE + len(s_extra)] = \
                tab[gids[s_extra]].T.astype(bf16)
        xs_c["s"] = xT
        swT = np.ascontiguousarray(sw.T)
        srce = np.ascontiguousarray(swT[:, kind]) if kind.any() else \
            np.zeros((128, 1), dtype=np.int32)
        core_edges.append((swT, srce, mte, mtd, xs_c))

    # ---- constants / weights ----
    identf_np = np.eye(128, dtype=np.float32)
    W1 = np.asarray(inputs["W1"], dtype=np.float32)
    b1 = np.asarray(inputs["b1"], dtype=np.float32)
    W2 = np.asarray(inputs["W2"], dtype=np.float32)
    b2 = np.asarray(inputs["b2"], dtype=np.float32)
    wh = np.asarray(inputs["wh"], dtype=np.float32)
    wh_b = np.asarray(inputs["wh_b"], dtype=np.float32)
    b1c = np.ascontiguousarray(b1.reshape(FFN // 128, 128).T)

    nc = _build(SLABS, slab_win, pc, kind.tolist())
    in_maps = []
    for c in range(NCORES):
        sw, srce, mte, mtd, xs_c = core_edges[c]
        m = {"iaa": iaa_np, "identf": identf_np,
             "src": sw, "srce": srce, "mte": mte, "mtd": mtd,
             "w1": W1.astype(bf16), "b1c": b1c, "w2": W2.astype(bf16),
             "b2c": np.ascontiguousarray(b2.reshape(128, 1)),
             "wh": wh.astype(bf16), "whb": wh_b.reshape(1, 1).astype(np.float32)}
        for t in ("s", "f", "u", "i"):
            m[f"xs_{t}"] = xs_c[t]
            m[f"wp_{t}"] = wp_np[t]
        in_maps.append(m)
    res, t1, amort = _run_spmd(nc, in_maps, NCORES, bench=bench)
    LAST_STATS["exec1_ns"] = int(t1 * 1e9) if t1 else None
    LAST_STATS["amort_ns"] = int(amort * 1e9) if amort else None
    LAST_STATS["nc"] = nc

    out = np.zeros(N_SENT, dtype=np.float32)
    ar128 = np.arange(128)
    for c in range(NCORES):
        lg = np.asarray(res[c]["logits"]).reshape(NW_CORE, 128)
        ids = (wperm[c][:, None] * 128 + ar128[None, :]).ravel()
        m = ids < N_SENT
        out[ids[m]] = lg.reshape(-1)[m]
    return np.ascontiguousarray(out.reshape(N_SENT, 1)).astype(np.float32)
